# revision 16
# baseline (speedup 1.0000x reference)
"""MoE (8 experts, top-2, d=1024, N=8192) on 8 trn2 NeuronCores.

Strategy (expert-parallel, per sharding hint):
 - Host computes routing (top-2 expert ids per token, fp64 logits for stable
   ordering) and dispatches: core e receives the tokens routed to expert e in
   a tiled layout xg_t [T, 128, 8, 128] (C = padded max expert load, T=C/128).
 - Device (per core, SPMD): router logits for its tokens (replicated router),
   top-2 gate g = sigmoid(2*l_own - m1 - m2), expert matmul y = (xg @ W[e])*g
   with PSUM K-accumulation in float32r (TF32-class, full-rate PE).
   Work is grouped G=6 token-tiles per DMA/gate-chain to amortize DMA-issue
   and DVE-issue overheads. Gates are also output for the host bias term.
 - Host combines: out[idx_e] += y_e + g_e*b[e]  (each token appears in exactly
   2 experts' index lists; indices unique within an expert).
"""

import os
from contextlib import ExitStack

import ml_dtypes
import numpy as np

import concourse.bass as bass
import concourse.bacc as bacc
import concourse.mybir as mybir
import concourse.tile as tile
from concourse.bass import ts
from concourse.bass_utils import run_bass_kernel_spmd

N_EXPERTS = 8
TOP_K = 2
D = 1024
N_CORES = 8
P = 128  # partitions
KT = D // P  # number of K tiles (8)
NH = 512  # psum free-dim tile (one bank of fp32)
EW = N_EXPERTS + 2  # logit row: 8 experts + own-logit col + pad col (even for fp32r)
G = 6  # token tiles per group

# matmul operand dtype for the expert/router matmuls:
#   "f32"  : plain fp32 (4 cycles/row)
#   "f32r" : float32r / TF32 (1 cycle/row at free dim >= 256)
#   "bf16" : bfloat16
MM_DTYPE = os.environ.get("MOE_MM_DTYPE", "f32r")

LAST_RESULTS = None  # stash of BassKernelResults for test harness inspection

_BUILD_CACHE = {}


def _build(C: int, repeat: int = 1):
    """Build the SPMD Bass module for per-core padded token count C."""
    key = (C, MM_DTYPE, repeat)
    if key in _BUILD_CACHE:
        return _BUILD_CACHE[key]

    f32 = mybir.dt.float32
    mm_dt = {
        "f32": mybir.dt.float32,
        "f32r": mybir.dt.float32r,
        "bf16": mybir.dt.bfloat16,
    }[MM_DTYPE]
    T = C // P

    nc = bacc.Bacc(None, target_bir_lowering=False)
    # inputs (xg_t: tiled tokens [T, 128 din-sub, KT, 128 tok])
    xg_t = nc.declare_dram_parameter("xg_t", [T, P, KT, P], mm_dt, isOutput=False)
    w = nc.declare_dram_parameter("w", [D, D], mm_dt, isOutput=False)
    wr = nc.declare_dram_parameter("wr", [D, EW], mm_dt, isOutput=False)
    brg = nc.declare_dram_parameter("brg", [P, G, EW], f32, isOutput=False)
    # outputs (y tiled [T, 2, 128 tok, 512]; gates [T, 128 tok])
    y = nc.declare_dram_parameter("y", [T, D // NH, P, NH], f32, isOutput=True)
    gout = nc.declare_dram_parameter("gout", [T, P], f32, isOutput=True)

    with tile.TileContext(nc) as tc, ExitStack() as ctx:
        consts = ctx.enter_context(tc.tile_pool(name="consts", bufs=1))
        xpool = ctx.enter_context(tc.tile_pool(name="x", bufs=2))
        gpool = ctx.enter_context(tc.tile_pool(name="gates", bufs=2))
        ypool = ctx.enter_context(tc.tile_pool(name="y", bufs=2))
        lpsum = ctx.enter_context(
            tc.tile_pool(name="lpsum", bufs=2, space=bass.MemorySpace.PSUM)
        )
        ypsum = ctx.enter_context(
            tc.tile_pool(name="ypsum", bufs=5, space=bass.MemorySpace.PSUM)
        )

        # ---- constants / weights resident in SBUF ----
        w_sb = consts.tile([P, KT, D], mm_dt)
        nc.sync.dma_start(w_sb[:], w.rearrange("(kt p) n -> p kt n", p=P))

        wr_sb = consts.tile([P, KT, EW], mm_dt)
        nc.sync.dma_start(wr_sb[:], wr.rearrange("(kt p) n -> p kt n", p=P))

        brg_sb = consts.tile([P, G, EW], f32)
        nc.sync.dma_start(brg_sb[:], brg[:, :, :])

        g_sb = consts.tile([P, T], f32)

        rep_cm = tc.For_i(0, repeat, 1) if repeat > 1 else None
        if rep_cm is not None:
            rep_cm.__enter__()

        for g0 in range(0, T, G):
            gt = min(G, T - g0)
            # ---- one DMA: gt token tiles [P, gt, KT, P] ----
            xt = xpool.tile([P, gt, KT, P], mm_dt, tag="xt")
            nc.sync.dma_start(
                xt[:], xg_t[g0 : g0 + gt].rearrange("g p j c -> p g j c")
            )

            # ---- router logits for the group: Lp [P, gt, EW] ----
            Lp = lpsum.tile([P, gt, EW], f32, tag="lp")
            for tau in range(gt):
                for j in range(KT):
                    nc.tensor.matmul(
                        Lp[:, tau, :],
                        xt[:, tau, j, :],
                        wr_sb[:, j, :],
                        start=(j == 0),
                        stop=(j == KT - 1),
                    )

            # ---- batched gate chain for the group ----
            La = gpool.tile([P, gt, EW], f32, tag="la")
            nc.vector.tensor_add(La[:], Lp[:], brg_sb[:, 0:gt, :])
            m1 = gpool.tile([P, gt, 1], f32, tag="m1")
            nc.vector.reduce_max(
                m1[:], La[:, :, 0:N_EXPERTS], axis=mybir.AxisListType.X
            )
            eq = gpool.tile([P, gt, N_EXPERTS], f32, tag="eq")
            nc.vector.tensor_tensor(
                eq[:],
                La[:, :, 0:N_EXPERTS],
                m1[:].broadcast_to((P, gt, N_EXPERTS)),
                mybir.AluOpType.is_equal,
            )
            nc.vector.tensor_scalar_mul(eq[:], eq[:], -1e30)
            nc.vector.tensor_add(eq[:], eq[:], La[:, :, 0:N_EXPERTS])
            m2 = gpool.tile([P, gt, 1], f32, tag="m2")
            nc.vector.reduce_max(m2[:], eq[:], axis=mybir.AxisListType.X)
            s = gpool.tile([P, gt, 1], f32, tag="s")
            nc.vector.tensor_scalar_mul(
                s[:], La[:, :, N_EXPERTS : N_EXPERTS + 1], 2.0
            )
            nc.vector.tensor_sub(s[:], s[:], m1[:])
            nc.vector.tensor_sub(s[:], s[:], m2[:])
            nc.scalar.activation(
                g_sb[:, g0 : g0 + gt],
                s[:, :, 0],
                mybir.ActivationFunctionType.Sigmoid,
            )

            # ---- expert matmuls + gate scale; one store DMA per group ----
            ysb = ypool.tile([P, gt, D // NH, NH], f32, tag="ysb")
            for tau in range(gt):
                for nh in range(D // NH):
                    yp = ypsum.tile([P, NH], f32, tag="yp")
                    for j in range(KT):
                        nc.tensor.matmul(
                            yp[:],
                            xt[:, tau, j, :],
                            w_sb[:, j, ts(nh, NH)],
                            start=(j == 0),
                            stop=(j == KT - 1),
                        )
                    nc.vector.tensor_scalar_mul(
                        ysb[:, tau, nh, :],
                        yp[:],
                        g_sb[:, g0 + tau : g0 + tau + 1],
                    )
            nc.sync.dma_start(
                y[g0 : g0 + gt].rearrange("g h p n -> p g h n"), ysb[:]
            )

        # gates out: one DMA
        nc.sync.dma_start(gout.rearrange("t p -> p t"), g_sb[:])

        if rep_cm is not None:
            rep_cm.__exit__(None, None, None)

    nc.compile()
    _BUILD_CACHE[key] = nc
    return nc


def _route(x, Wr, br):
    """Host routing in fp64: per-token top-2 expert ids."""
    n_tokens = x.shape[0]
    logits = x.astype(np.float64) @ Wr.astype(np.float64) + br.astype(np.float64)
    i1 = np.argmax(logits, axis=1)
    l2 = logits.copy()
    l2[np.arange(n_tokens), i1] = -np.inf
    i2 = np.argmax(l2, axis=1)
    return i1, i2


def _make_in_maps(x, Wr, br, W, b, idx_per_e, C):
    np_mm = ml_dtypes.bfloat16 if MM_DTYPE == "bf16" else np.float32
    T = C // P
    in_maps = []
    for e in range(N_CORES):
        idx = idx_per_e[e]
        xg = np.zeros((C, D), dtype=np.float32)
        xg[: len(idx)] = x[idx]
        # tiled layout: xg_t[t, p, j, c] = xg[t*128 + c, j*128 + p]
        xg_t = np.ascontiguousarray(
            xg.reshape(T, P, KT, P).transpose(0, 3, 2, 1)
        ).astype(np_mm)
        zcol = np.zeros((D, 1), dtype=np.float32)
        wr = np.concatenate([Wr, Wr[:, e : e + 1], zcol], axis=1)
        brv = np.concatenate(
            [br, br[e : e + 1], np.zeros(1, np.float32)]
        ).astype(np.float32)
        in_maps.append(
            {
                "xg_t": xg_t,
                "w": np.ascontiguousarray(W[e]).astype(np_mm),
                "wr": np.ascontiguousarray(wr).astype(np_mm),
                "brg": np.broadcast_to(brv[None, None, :], (P, G, EW)).copy(),
            }
        )
    return in_maps


def _prep(inputs):
    x = np.asarray(inputs["x"], dtype=np.float32)
    Wr = np.asarray(inputs["Wr"], dtype=np.float32)
    br = np.asarray(inputs["br"], dtype=np.float32)
    W = np.asarray(inputs["W"], dtype=np.float32)
    b = np.asarray(inputs["b"], dtype=np.float32)
    i1, i2 = _route(x, Wr, br)
    idx_per_e = [np.where((i1 == e) | (i2 == e))[0] for e in range(N_EXPERTS)]
    C = max(P, ((max(len(ix) for ix in idx_per_e) + P - 1) // P) * P)
    in_maps = _make_in_maps(x, Wr, br, W, b, idx_per_e, C)
    return in_maps, idx_per_e, C, x.shape[0], b


def kernel(**inputs) -> np.ndarray:
    global LAST_RESULTS
    in_maps, idx_per_e, C, n_tokens, b = _prep(inputs)
    T = C // P
    nc = _build(C)
    res = run_bass_kernel_spmd(nc, in_maps, core_ids=list(range(N_CORES)))
    LAST_RESULTS = res

    out = np.zeros((n_tokens, D), dtype=np.float32)
    for e in range(N_CORES):
        idx = idx_per_e[e]
        n = len(idx)
        # y [T, 2, P, NH] -> [C, D]
        ye = res.results[e]["y"].transpose(0, 2, 1, 3).reshape(C, D)
        ge = res.results[e]["gout"].reshape(C)
        out[idx] += ye[:n] + ge[:n, None] * b[e][None, :]
    return out


# revision 20
# speedup vs baseline: 1.0477x; 1.0477x over previous
"""MoE (8 experts, top-2, d=1024, N=8192) on 8 trn2 NeuronCores.

Strategy (expert-parallel, per sharding hint):
 - Host computes routing (top-2 expert ids per token, fp64 logits for stable
   ordering) and dispatches: core e receives the tokens routed to expert e in
   a tiled layout xg_t [T, 128, 8, 128] (C = padded max expert load, T=C/128).
 - Device (per core, SPMD): router logits for its tokens (replicated router),
   top-2 gate g = sigmoid(2*l_own - m1 - m2), expert matmul y = (xg @ W[e])*g
   with PSUM K-accumulation in float32r (TF32-class, full-rate PE).
   Work is grouped G=6 token-tiles per DMA/gate-chain to amortize DMA-issue
   and DVE-issue overheads. Gates are also output for the host bias term.
 - Host combines: out[idx_e] += y_e + g_e*b[e]  (each token appears in exactly
   2 experts' index lists; indices unique within an expert).
"""

import os
from contextlib import ExitStack

import ml_dtypes
import numpy as np

import concourse.bass as bass
import concourse.bacc as bacc
import concourse.mybir as mybir
import concourse.tile as tile
from concourse.bass import ts
from concourse.bass_utils import run_bass_kernel_spmd

N_EXPERTS = 8
TOP_K = 2
D = 1024
N_CORES = 8
P = 128  # partitions
KT = D // P  # number of K tiles (8)
NH = 512  # psum free-dim tile (one bank of fp32)
EW = N_EXPERTS + 2  # logit row: 8 experts + own-logit col + pad col (even for fp32r)
G = 6  # token tiles per group

# matmul operand dtype for the expert/router matmuls:
#   "f32"  : plain fp32 (4 cycles/row)
#   "f32r" : float32r / TF32 (1 cycle/row at free dim >= 256)
#   "bf16" : bfloat16
MM_DTYPE = os.environ.get("MOE_MM_DTYPE", "f32r")

LAST_RESULTS = None  # stash of BassKernelResults for test harness inspection

_BUILD_CACHE = {}


def _build(C: int, repeat: int = 1):
    """Build the SPMD Bass module for per-core padded token count C."""
    key = (C, MM_DTYPE, repeat)
    if key in _BUILD_CACHE:
        return _BUILD_CACHE[key]

    f32 = mybir.dt.float32
    mm_dt = {
        "f32": mybir.dt.float32,
        "f32r": mybir.dt.float32r,
        "bf16": mybir.dt.bfloat16,
    }[MM_DTYPE]
    T = C // P

    nc = bacc.Bacc(None, target_bir_lowering=False)
    # inputs (xg_t: tiled tokens [T, 128 din-sub, KT, 128 tok])
    xg_t = nc.declare_dram_parameter("xg_t", [P, T * KT * P], mm_dt, isOutput=False)
    w = nc.declare_dram_parameter("w", [D, D], mm_dt, isOutput=False)
    wr = nc.declare_dram_parameter("wr", [D, EW], mm_dt, isOutput=False)
    # mask+bias row: br[c] for other experts, -1e30 at own/8/9 cols (tiled x G)
    mb = nc.declare_dram_parameter("mb", [P, G * EW], f32, isOutput=False)
    brown = nc.declare_dram_parameter("brown", [P, 1], f32, isOutput=False)
    # outputs (y tiled [T, 2, 128 tok, 512]; gates [T, 128 tok])
    y = nc.declare_dram_parameter("y", [P, T * D], f32, isOutput=True)
    gout = nc.declare_dram_parameter("gout", [P, T], f32, isOutput=True)

    with tile.TileContext(nc) as tc, ExitStack() as ctx:
        consts = ctx.enter_context(tc.tile_pool(name="consts", bufs=1))
        xpool = ctx.enter_context(tc.tile_pool(name="x", bufs=3))
        gpool = ctx.enter_context(tc.tile_pool(name="gates", bufs=2))
        ypool = ctx.enter_context(tc.tile_pool(name="y", bufs=3))
        lpsum = ctx.enter_context(
            tc.tile_pool(name="lpsum", bufs=2, space=bass.MemorySpace.PSUM)
        )
        ypsum = ctx.enter_context(
            tc.tile_pool(name="ypsum", bufs=3, space=bass.MemorySpace.PSUM)
        )

        # ---- constants / weights resident in SBUF ----
        w_sb = consts.tile([P, KT, D], mm_dt)
        nc.sync.dma_start(w_sb[:], w.rearrange("(kt p) n -> p kt n", p=P))

        wr_sb = consts.tile([P, KT, EW], mm_dt)
        nc.sync.dma_start(wr_sb[:], wr.rearrange("(kt p) n -> p kt n", p=P))

        mb_sb = consts.tile([P, G * EW], f32)
        nc.sync.dma_start(mb_sb[:], mb[:, :])
        brown_sb = consts.tile([P, 1], f32)
        nc.sync.dma_start(brown_sb[:], brown[:, :])

        rep_cm = tc.For_i(0, repeat, 1) if repeat > 1 else None
        if rep_cm is not None:
            rep_cm.__enter__()

        groups = [(g0, min(G, T - g0)) for g0 in range(0, T, G)]

        def load_x(g0, gt):
            xt = xpool.tile([P, gt * KT * P], mm_dt, tag="xt")
            nc.sync.dma_start(xt[:], xg_t[:, g0 * KT * P : (g0 + gt) * KT * P])
            return xt

        def router_gates(xt, gt):
            """Router matmuls + gate chain: g = sigmoid(l_own - max_{e!=own})."""
            Lp = lpsum.tile([P, gt * EW], f32, tag="lp")
            for tau in range(gt):
                for j in range(KT):
                    nc.tensor.matmul(
                        Lp[:, tau * EW : (tau + 1) * EW],
                        xt[:, (tau * KT + j) * P : (tau * KT + j + 1) * P],
                        wr_sb[:, j, :],
                        start=(j == 0),
                        stop=(j == KT - 1),
                    )
            Lm = gpool.tile([P, gt * EW], f32, tag="lm")
            nc.vector.tensor_add(Lm[:], Lp[:], mb_sb[:, 0 : gt * EW])
            mo = gpool.tile([P, gt, 1], f32, tag="mo")
            nc.vector.reduce_max(
                mo[:],
                Lm[:].rearrange("p (g e) -> p g e", e=EW),
                axis=mybir.AxisListType.X,
            )
            sg = gpool.tile([P, gt, 1], f32, tag="sg")
            Lp3 = Lp[:].rearrange("p (g e) -> p g e", e=EW)
            nc.vector.tensor_scalar_add(
                sg[:], Lp3[:, :, N_EXPERTS : N_EXPERTS + 1], brown_sb[:]
            )
            nc.vector.tensor_sub(sg[:], sg[:], mo[:])
            gg = gpool.tile([P, gt], f32, tag="gg")
            nc.scalar.activation(
                gg[:], sg[:, :, 0], mybir.ActivationFunctionType.Sigmoid
            )
            return gg

        def experts(g0, gt, xt, gg):
            ysb = ypool.tile([P, gt * D], f32, tag="ysb")
            for tau in range(gt):
                yp = ypsum.tile([P, D], f32, tag="yp")
                for nh in range(D // NH):
                    for j in range(KT):
                        nc.tensor.matmul(
                            yp[:, ts(nh, NH)],
                            xt[:, (tau * KT + j) * P : (tau * KT + j + 1) * P],
                            w_sb[:, j, ts(nh, NH)],
                            start=(j == 0),
                            stop=(j == KT - 1),
                        )
                dst = ysb[:, tau * D : (tau + 1) * D]
                gsc = gg[:, tau : tau + 1]
                if tau % 2 == 0:
                    nc.vector.tensor_scalar_mul(dst, yp[:], gsc)
                else:
                    nc.scalar.mul(dst, yp[:], gsc)
            nc.sync.dma_start(y[:, g0 * D : (g0 + gt) * D], ysb[:])
            nc.sync.dma_start(gout[:, g0 : g0 + gt], gg[:])

        # software pipeline: router+gates run one group ahead of experts
        xt_cur = load_x(*groups[0])
        gg_cur = router_gates(xt_cur, groups[0][1])
        for i, (g0, gt) in enumerate(groups):
            if i + 1 < len(groups):
                xt_nxt = load_x(*groups[i + 1])
                gg_nxt = router_gates(xt_nxt, groups[i + 1][1])
            experts(g0, gt, xt_cur, gg_cur)
            if i + 1 < len(groups):
                xt_cur, gg_cur = xt_nxt, gg_nxt

        if rep_cm is not None:
            rep_cm.__exit__(None, None, None)

    nc.compile()
    _BUILD_CACHE[key] = nc
    return nc


def _route(x, Wr, br):
    """Host routing in fp64: per-token top-2 expert ids."""
    n_tokens = x.shape[0]
    logits = x.astype(np.float64) @ Wr.astype(np.float64) + br.astype(np.float64)
    i1 = np.argmax(logits, axis=1)
    l2 = logits.copy()
    l2[np.arange(n_tokens), i1] = -np.inf
    i2 = np.argmax(l2, axis=1)
    return i1, i2


def _make_in_maps(x, Wr, br, W, b, idx_per_e, C):
    np_mm = ml_dtypes.bfloat16 if MM_DTYPE == "bf16" else np.float32
    T = C // P
    in_maps = []
    for e in range(N_CORES):
        idx = idx_per_e[e]
        xg = np.zeros((C, D), dtype=np.float32)
        xg[: len(idx)] = x[idx]
        # partition-major layout: xg_t[p, t, j, c] = xg[t*128 + c, j*128 + p]
        xg_t = np.ascontiguousarray(
            xg.reshape(T, P, KT, P).transpose(3, 0, 2, 1).reshape(P, T * KT * P)
        ).astype(np_mm)
        zcol = np.zeros((D, 1), dtype=np.float32)
        wr = np.concatenate([Wr, Wr[:, e : e + 1], zcol], axis=1)
        mbrow = np.concatenate(
            [br, np.full(2, -1e30, np.float32)]
        ).astype(np.float32)
        mbrow[e] = -1e30
        mbv = np.tile(mbrow, G)
        in_maps.append(
            {
                "xg_t": xg_t,
                "w": np.ascontiguousarray(W[e]).astype(np_mm),
                "wr": np.ascontiguousarray(wr).astype(np_mm),
                "mb": np.broadcast_to(mbv[None, :], (P, G * EW)).copy(),
                "brown": np.full((P, 1), br[e], dtype=np.float32),
            }
        )
    return in_maps


def _prep(inputs):
    x = np.asarray(inputs["x"], dtype=np.float32)
    Wr = np.asarray(inputs["Wr"], dtype=np.float32)
    br = np.asarray(inputs["br"], dtype=np.float32)
    W = np.asarray(inputs["W"], dtype=np.float32)
    b = np.asarray(inputs["b"], dtype=np.float32)
    i1, i2 = _route(x, Wr, br)
    idx_per_e = [np.where((i1 == e) | (i2 == e))[0] for e in range(N_EXPERTS)]
    C = max(P, ((max(len(ix) for ix in idx_per_e) + P - 1) // P) * P)
    in_maps = _make_in_maps(x, Wr, br, W, b, idx_per_e, C)
    return in_maps, idx_per_e, C, x.shape[0], b


def kernel(**inputs) -> np.ndarray:
    global LAST_RESULTS
    in_maps, idx_per_e, C, n_tokens, b = _prep(inputs)
    T = C // P
    nc = _build(C)
    res = run_bass_kernel_spmd(nc, in_maps, core_ids=list(range(N_CORES)))
    LAST_RESULTS = res

    out = np.zeros((n_tokens, D), dtype=np.float32)
    for e in range(N_CORES):
        idx = idx_per_e[e]
        n = len(idx)
        # y [P, T*D]: y[p, t*D + f] = token (t*128+p), feature f
        ye = (
            res.results[e]["y"].reshape(P, T, D).transpose(1, 0, 2).reshape(C, D)
        )
        ge = res.results[e]["gout"].T.reshape(C)
        out[idx] += ye[:n] + ge[:n, None] * b[e][None, :]
    return out


# revision 21
# speedup vs baseline: 1.7385x; 1.6594x over previous
"""MoE (8 experts, top-2, d=1024, N=8192) on 8 trn2 NeuronCores.

Strategy (expert-parallel, per sharding hint):
 - Host computes routing (top-2 expert ids per token, fp64 logits for stable
   ordering) and dispatches: core e receives the tokens routed to expert e in
   a tiled layout xg_t [T, 128, 8, 128] (C = padded max expert load, T=C/128).
 - Device (per core, SPMD): router logits for its tokens (replicated router),
   top-2 gate g = sigmoid(2*l_own - m1 - m2), expert matmul y = (xg @ W[e])*g
   with PSUM K-accumulation in float32r (TF32-class, full-rate PE).
   Work is grouped G=6 token-tiles per DMA/gate-chain to amortize DMA-issue
   and DVE-issue overheads. Gates are also output for the host bias term.
 - Host combines: out[idx_e] += y_e + g_e*b[e]  (each token appears in exactly
   2 experts' index lists; indices unique within an expert).
"""

import os
from contextlib import ExitStack

import ml_dtypes
import numpy as np

import concourse.bass as bass
import concourse.bacc as bacc
import concourse.mybir as mybir
import concourse.tile as tile
from concourse.bass import ts
from concourse.bass_utils import run_bass_kernel_spmd

N_EXPERTS = 8
TOP_K = 2
D = 1024
N_CORES = 8
P = 128  # partitions
KT = D // P  # number of K tiles (8)
NH = 512  # psum free-dim tile (one bank of fp32)
EW = N_EXPERTS + 2  # logit row: 8 experts + own-logit col + pad col (even for fp32r)
G = int(os.environ.get("MOE_G", "1"))  # token tiles per group

# matmul operand dtype for the expert/router matmuls:
#   "f32"  : plain fp32 (4 cycles/row)
#   "f32r" : float32r / TF32 (1 cycle/row at free dim >= 256)
#   "bf16" : bfloat16
MM_DTYPE = os.environ.get("MOE_MM_DTYPE", "f32r")

LAST_RESULTS = None  # stash of BassKernelResults for test harness inspection

_BUILD_CACHE = {}


def _build(C: int, repeat: int = 1):
    """Build the SPMD Bass module for per-core padded token count C."""
    key = (C, MM_DTYPE, repeat, G)
    if key in _BUILD_CACHE:
        return _BUILD_CACHE[key]

    f32 = mybir.dt.float32
    mm_dt = {
        "f32": mybir.dt.float32,
        "f32r": mybir.dt.float32r,
        "bf16": mybir.dt.bfloat16,
    }[MM_DTYPE]
    T = C // P

    nc = bacc.Bacc(None, target_bir_lowering=False)
    # inputs (xg_t: tiled tokens [T, 128 din-sub, KT, 128 tok])
    xg_t = nc.declare_dram_parameter("xg_t", [P, T * KT * P], mm_dt, isOutput=False)
    w = nc.declare_dram_parameter("w", [D, D], mm_dt, isOutput=False)
    wr = nc.declare_dram_parameter("wr", [D, EW], mm_dt, isOutput=False)
    # mask+bias row: br[c] for other experts, -1e30 at own/8/9 cols (tiled x G)
    mb = nc.declare_dram_parameter("mb", [P, G * EW], f32, isOutput=False)
    brown = nc.declare_dram_parameter("brown", [P, 1], f32, isOutput=False)
    # outputs (y tiled [T, 2, 128 tok, 512]; gates [T, 128 tok])
    y = nc.declare_dram_parameter("y", [P, T * D], f32, isOutput=True)
    gout = nc.declare_dram_parameter("gout", [P, T], f32, isOutput=True)

    with tile.TileContext(nc) as tc, ExitStack() as ctx:
        consts = ctx.enter_context(tc.tile_pool(name="consts", bufs=1))
        xpool = ctx.enter_context(tc.tile_pool(name="x", bufs=3))
        gpool = ctx.enter_context(tc.tile_pool(name="gates", bufs=2))
        ypool = ctx.enter_context(tc.tile_pool(name="y", bufs=3))
        lpsum = ctx.enter_context(
            tc.tile_pool(name="lpsum", bufs=2, space=bass.MemorySpace.PSUM)
        )
        ypsum = ctx.enter_context(
            tc.tile_pool(name="ypsum", bufs=3, space=bass.MemorySpace.PSUM)
        )

        # ---- constants / weights resident in SBUF ----
        w_sb = consts.tile([P, KT, D], mm_dt)
        nc.sync.dma_start(w_sb[:], w.rearrange("(kt p) n -> p kt n", p=P))

        wr_sb = consts.tile([P, KT, EW], mm_dt)
        nc.sync.dma_start(wr_sb[:], wr.rearrange("(kt p) n -> p kt n", p=P))

        mb_sb = consts.tile([P, G * EW], f32)
        nc.sync.dma_start(mb_sb[:], mb[:, :])
        brown_sb = consts.tile([P, 1], f32)
        nc.sync.dma_start(brown_sb[:], brown[:, :])

        rep_cm = tc.For_i(0, repeat, 1) if repeat > 1 else None
        if rep_cm is not None:
            rep_cm.__enter__()

        groups = [(g0, min(G, T - g0)) for g0 in range(0, T, G)]

        def load_x(g0, gt):
            xt = xpool.tile([P, gt * KT * P], mm_dt, tag="xt")
            nc.sync.dma_start(xt[:], xg_t[:, g0 * KT * P : (g0 + gt) * KT * P])
            return xt

        def router_gates(xt, gt):
            """Router matmuls + gate chain: g = sigmoid(l_own - max_{e!=own})."""
            Lp = lpsum.tile([P, gt * EW], f32, tag="lp")
            for tau in range(gt):
                for j in range(KT):
                    nc.tensor.matmul(
                        Lp[:, tau * EW : (tau + 1) * EW],
                        xt[:, (tau * KT + j) * P : (tau * KT + j + 1) * P],
                        wr_sb[:, j, :],
                        start=(j == 0),
                        stop=(j == KT - 1),
                    )
            Lm = gpool.tile([P, gt * EW], f32, tag="lm")
            nc.vector.tensor_add(Lm[:], Lp[:], mb_sb[:, 0 : gt * EW])
            mo = gpool.tile([P, gt, 1], f32, tag="mo")
            nc.vector.reduce_max(
                mo[:],
                Lm[:].rearrange("p (g e) -> p g e", e=EW),
                axis=mybir.AxisListType.X,
            )
            sg = gpool.tile([P, gt, 1], f32, tag="sg")
            Lp3 = Lp[:].rearrange("p (g e) -> p g e", e=EW)
            nc.vector.tensor_scalar_add(
                sg[:], Lp3[:, :, N_EXPERTS : N_EXPERTS + 1], brown_sb[:]
            )
            nc.vector.tensor_sub(sg[:], sg[:], mo[:])
            gg = gpool.tile([P, gt], f32, tag="gg")
            nc.scalar.activation(
                gg[:], sg[:, :, 0], mybir.ActivationFunctionType.Sigmoid
            )
            return gg

        def experts(g0, gt, xt, gg):
            ysb = ypool.tile([P, gt * D], f32, tag="ysb")
            for tau in range(gt):
                yp = ypsum.tile([P, D], f32, tag="yp")
                for nh in range(D // NH):
                    for j in range(KT):
                        nc.tensor.matmul(
                            yp[:, ts(nh, NH)],
                            xt[:, (tau * KT + j) * P : (tau * KT + j + 1) * P],
                            w_sb[:, j, ts(nh, NH)],
                            start=(j == 0),
                            stop=(j == KT - 1),
                        )
                dst = ysb[:, tau * D : (tau + 1) * D]
                gsc = gg[:, tau : tau + 1]
                if tau % 2 == 0:
                    nc.vector.tensor_scalar_mul(dst, yp[:], gsc)
                else:
                    nc.scalar.mul(dst, yp[:], gsc)
            nc.sync.dma_start(y[:, g0 * D : (g0 + gt) * D], ysb[:])
            nc.sync.dma_start(gout[:, g0 : g0 + gt], gg[:])

        # software pipeline: router+gates run one group ahead of experts
        xt_cur = load_x(*groups[0])
        gg_cur = router_gates(xt_cur, groups[0][1])
        for i, (g0, gt) in enumerate(groups):
            if i + 1 < len(groups):
                xt_nxt = load_x(*groups[i + 1])
                gg_nxt = router_gates(xt_nxt, groups[i + 1][1])
            experts(g0, gt, xt_cur, gg_cur)
            if i + 1 < len(groups):
                xt_cur, gg_cur = xt_nxt, gg_nxt

        if rep_cm is not None:
            rep_cm.__exit__(None, None, None)

    nc.compile()
    _BUILD_CACHE[key] = nc
    return nc


def _route(x, Wr, br):
    """Host routing in fp64: per-token top-2 expert ids."""
    n_tokens = x.shape[0]
    logits = x.astype(np.float64) @ Wr.astype(np.float64) + br.astype(np.float64)
    i1 = np.argmax(logits, axis=1)
    l2 = logits.copy()
    l2[np.arange(n_tokens), i1] = -np.inf
    i2 = np.argmax(l2, axis=1)
    return i1, i2


def _make_in_maps(x, Wr, br, W, b, idx_per_e, C):
    np_mm = ml_dtypes.bfloat16 if MM_DTYPE == "bf16" else np.float32
    T = C // P
    in_maps = []
    for e in range(N_CORES):
        idx = idx_per_e[e]
        xg = np.zeros((C, D), dtype=np.float32)
        xg[: len(idx)] = x[idx]
        # partition-major layout: xg_t[p, t, j, c] = xg[t*128 + c, j*128 + p]
        xg_t = np.ascontiguousarray(
            xg.reshape(T, P, KT, P).transpose(3, 0, 2, 1).reshape(P, T * KT * P)
        ).astype(np_mm)
        zcol = np.zeros((D, 1), dtype=np.float32)
        wr = np.concatenate([Wr, Wr[:, e : e + 1], zcol], axis=1)
        mbrow = np.concatenate(
            [br, np.full(2, -1e30, np.float32)]
        ).astype(np.float32)
        mbrow[e] = -1e30
        mbv = np.tile(mbrow, G)
        in_maps.append(
            {
                "xg_t": xg_t,
                "w": np.ascontiguousarray(W[e]).astype(np_mm),
                "wr": np.ascontiguousarray(wr).astype(np_mm),
                "mb": np.broadcast_to(mbv[None, :], (P, G * EW)).copy(),
                "brown": np.full((P, 1), br[e], dtype=np.float32),
            }
        )
    return in_maps


def _prep(inputs):
    x = np.asarray(inputs["x"], dtype=np.float32)
    Wr = np.asarray(inputs["Wr"], dtype=np.float32)
    br = np.asarray(inputs["br"], dtype=np.float32)
    W = np.asarray(inputs["W"], dtype=np.float32)
    b = np.asarray(inputs["b"], dtype=np.float32)
    i1, i2 = _route(x, Wr, br)
    idx_per_e = [np.where((i1 == e) | (i2 == e))[0] for e in range(N_EXPERTS)]
    C = max(P, ((max(len(ix) for ix in idx_per_e) + P - 1) // P) * P)
    in_maps = _make_in_maps(x, Wr, br, W, b, idx_per_e, C)
    return in_maps, idx_per_e, C, x.shape[0], b


def kernel(**inputs) -> np.ndarray:
    global LAST_RESULTS
    in_maps, idx_per_e, C, n_tokens, b = _prep(inputs)
    T = C // P
    nc = _build(C)
    res = run_bass_kernel_spmd(nc, in_maps, core_ids=list(range(N_CORES)))
    LAST_RESULTS = res

    out = np.zeros((n_tokens, D), dtype=np.float32)
    for e in range(N_CORES):
        idx = idx_per_e[e]
        n = len(idx)
        # y [P, T*D]: y[p, t*D + f] = token (t*128+p), feature f
        ye = (
            res.results[e]["y"].reshape(P, T, D).transpose(1, 0, 2).reshape(C, D)
        )
        ge = res.results[e]["gout"].T.reshape(C)
        out[idx] += ye[:n] + ge[:n, None] * b[e][None, :]
    return out


# revision 22
# speedup vs baseline: 1.8561x; 1.0677x over previous
"""MoE (8 experts, top-2, d=1024, N=8192) on 8 trn2 NeuronCores.

Strategy (expert-parallel, per sharding hint):
 - Host computes routing (top-2 expert ids per token, fp64 logits for stable
   ordering) and dispatches: core e receives the tokens routed to expert e in
   a tiled layout xg_t [T, 128, 8, 128] (C = padded max expert load, T=C/128).
 - Device (per core, SPMD): router logits for its tokens (replicated router),
   top-2 gate g = sigmoid(2*l_own - m1 - m2), expert matmul y = (xg @ W[e])*g
   with PSUM K-accumulation in float32r (TF32-class, full-rate PE).
   Work is grouped G=3 token-tiles per DMA/gate-chain to amortize DMA-issue
   and DVE-issue overheads. Gates are also output for the host bias term.
 - Host combines: out[idx_e] += y_e + g_e*b[e]  (each token appears in exactly
   2 experts' index lists; indices unique within an expert).
"""

import os
from contextlib import ExitStack

import ml_dtypes
import numpy as np

import concourse.bass as bass
import concourse.bacc as bacc
import concourse.mybir as mybir
import concourse.tile as tile
from concourse.bass import ts
from concourse.bass_utils import run_bass_kernel_spmd

N_EXPERTS = 8
TOP_K = 2
D = 1024
N_CORES = 8
P = 128  # partitions
KT = D // P  # number of K tiles (8)
NH = 512  # psum free-dim tile (one bank of fp32)
EW = N_EXPERTS + 2  # logit row: 8 experts + own-logit col + pad col (even for fp32r)
G = int(os.environ.get("MOE_G", "3"))  # token tiles per group

# matmul operand dtype for the expert/router matmuls:
#   "f32"  : plain fp32 (4 cycles/row)
#   "f32r" : float32r / TF32 (1 cycle/row at free dim >= 256)
#   "bf16" : bfloat16
MM_DTYPE = os.environ.get("MOE_MM_DTYPE", "f32r")

LAST_RESULTS = None  # stash of BassKernelResults for test harness inspection

_BUILD_CACHE = {}


def _build(C: int, repeat: int = 1):
    """Build the SPMD Bass module for per-core padded token count C."""
    key = (C, MM_DTYPE, repeat, G)
    if key in _BUILD_CACHE:
        return _BUILD_CACHE[key]

    f32 = mybir.dt.float32
    mm_dt = {
        "f32": mybir.dt.float32,
        "f32r": mybir.dt.float32r,
        "bf16": mybir.dt.bfloat16,
    }[MM_DTYPE]
    T = C // P

    nc = bacc.Bacc(None, target_bir_lowering=False)
    # inputs (xg_t: tiled tokens [T, 128 din-sub, KT, 128 tok])
    xg_t = nc.declare_dram_parameter("xg_t", [P, T * KT * P], mm_dt, isOutput=False)
    w = nc.declare_dram_parameter("w", [D, D], mm_dt, isOutput=False)
    wr = nc.declare_dram_parameter("wr", [D, EW], mm_dt, isOutput=False)
    # mask+bias row: br[c] for other experts, -1e30 at own/8/9 cols (tiled x G)
    mb = nc.declare_dram_parameter("mb", [P, G * EW], f32, isOutput=False)
    brown = nc.declare_dram_parameter("brown", [P, 1], f32, isOutput=False)
    # outputs (y tiled [T, 2, 128 tok, 512]; gates [T, 128 tok])
    y = nc.declare_dram_parameter("y", [P, T * D], f32, isOutput=True)
    gout = nc.declare_dram_parameter("gout", [P, T], f32, isOutput=True)

    with tile.TileContext(nc) as tc, ExitStack() as ctx:
        consts = ctx.enter_context(tc.tile_pool(name="consts", bufs=1))
        xpool = ctx.enter_context(tc.tile_pool(name="x", bufs=3))
        gpool = ctx.enter_context(tc.tile_pool(name="gates", bufs=2))
        ypool = ctx.enter_context(tc.tile_pool(name="y", bufs=3))
        lpsum = ctx.enter_context(
            tc.tile_pool(name="lpsum", bufs=2, space=bass.MemorySpace.PSUM)
        )
        ypsum = ctx.enter_context(
            tc.tile_pool(name="ypsum", bufs=3, space=bass.MemorySpace.PSUM)
        )

        # ---- constants / weights resident in SBUF ----
        w_sb = consts.tile([P, KT, D], mm_dt)
        nc.sync.dma_start(w_sb[:], w.rearrange("(kt p) n -> p kt n", p=P))

        wr_sb = consts.tile([P, KT, EW], mm_dt)
        nc.sync.dma_start(wr_sb[:], wr.rearrange("(kt p) n -> p kt n", p=P))

        mb_sb = consts.tile([P, G * EW], f32)
        nc.sync.dma_start(mb_sb[:], mb[:, :])
        brown_sb = consts.tile([P, 1], f32)
        nc.sync.dma_start(brown_sb[:], brown[:, :])

        rep_cm = tc.For_i(0, repeat, 1) if repeat > 1 else None
        if rep_cm is not None:
            rep_cm.__enter__()

        groups = [(g0, min(G, T - g0)) for g0 in range(0, T, G)]

        def load_x(g0, gt):
            xt = xpool.tile([P, gt * KT * P], mm_dt, tag="xt")
            nc.sync.dma_start(xt[:], xg_t[:, g0 * KT * P : (g0 + gt) * KT * P])
            return xt

        def router_gates(xt, gt):
            """Router matmuls + gate chain: g = sigmoid(l_own - max_{e!=own})."""
            Lp = lpsum.tile([P, gt * EW], f32, tag="lp")
            for tau in range(gt):
                for j in range(KT):
                    nc.tensor.matmul(
                        Lp[:, tau * EW : (tau + 1) * EW],
                        xt[:, (tau * KT + j) * P : (tau * KT + j + 1) * P],
                        wr_sb[:, j, :],
                        start=(j == 0),
                        stop=(j == KT - 1),
                    )
            Lm = gpool.tile([P, gt * EW], f32, tag="lm")
            nc.vector.tensor_add(Lm[:], Lp[:], mb_sb[:, 0 : gt * EW])
            mo = gpool.tile([P, gt, 1], f32, tag="mo")
            nc.vector.reduce_max(
                mo[:],
                Lm[:].rearrange("p (g e) -> p g e", e=EW),
                axis=mybir.AxisListType.X,
            )
            sg = gpool.tile([P, gt, 1], f32, tag="sg")
            Lp3 = Lp[:].rearrange("p (g e) -> p g e", e=EW)
            nc.vector.tensor_scalar_add(
                sg[:], Lp3[:, :, N_EXPERTS : N_EXPERTS + 1], brown_sb[:]
            )
            nc.vector.tensor_sub(sg[:], sg[:], mo[:])
            gg = gpool.tile([P, gt], f32, tag="gg")
            nc.scalar.activation(
                gg[:], sg[:, :, 0], mybir.ActivationFunctionType.Sigmoid
            )
            return gg

        def experts(g0, gt, xt, gg):
            ysb = ypool.tile([P, gt * D], f32, tag="ysb")
            for tau in range(gt):
                yp = ypsum.tile([P, D], f32, tag="yp")
                for nh in range(D // NH):
                    for j in range(KT):
                        nc.tensor.matmul(
                            yp[:, ts(nh, NH)],
                            xt[:, (tau * KT + j) * P : (tau * KT + j + 1) * P],
                            w_sb[:, j, ts(nh, NH)],
                            start=(j == 0),
                            stop=(j == KT - 1),
                        )
                dst = ysb[:, tau * D : (tau + 1) * D]
                gsc = gg[:, tau : tau + 1]
                if tau % 2 == 0:
                    nc.vector.tensor_scalar_mul(dst, yp[:], gsc)
                else:
                    nc.scalar.mul(dst, yp[:], gsc)
            nc.sync.dma_start(y[:, g0 * D : (g0 + gt) * D], ysb[:])
            nc.sync.dma_start(gout[:, g0 : g0 + gt], gg[:])

        # software pipeline: router+gates run one group ahead of experts
        xt_cur = load_x(*groups[0])
        gg_cur = router_gates(xt_cur, groups[0][1])
        for i, (g0, gt) in enumerate(groups):
            if i + 1 < len(groups):
                xt_nxt = load_x(*groups[i + 1])
                gg_nxt = router_gates(xt_nxt, groups[i + 1][1])
            experts(g0, gt, xt_cur, gg_cur)
            if i + 1 < len(groups):
                xt_cur, gg_cur = xt_nxt, gg_nxt

        if rep_cm is not None:
            rep_cm.__exit__(None, None, None)

    nc.compile()
    _BUILD_CACHE[key] = nc
    return nc


def _route(x, Wr, br):
    """Host routing in fp64: per-token top-2 expert ids."""
    n_tokens = x.shape[0]
    logits = x.astype(np.float64) @ Wr.astype(np.float64) + br.astype(np.float64)
    i1 = np.argmax(logits, axis=1)
    l2 = logits.copy()
    l2[np.arange(n_tokens), i1] = -np.inf
    i2 = np.argmax(l2, axis=1)
    return i1, i2


def _make_in_maps(x, Wr, br, W, b, idx_per_e, C):
    np_mm = ml_dtypes.bfloat16 if MM_DTYPE == "bf16" else np.float32
    T = C // P
    in_maps = []
    for e in range(N_CORES):
        idx = idx_per_e[e]
        xg = np.zeros((C, D), dtype=np.float32)
        xg[: len(idx)] = x[idx]
        # partition-major layout: xg_t[p, t, j, c] = xg[t*128 + c, j*128 + p]
        xg_t = np.ascontiguousarray(
            xg.reshape(T, P, KT, P).transpose(3, 0, 2, 1).reshape(P, T * KT * P)
        ).astype(np_mm)
        zcol = np.zeros((D, 1), dtype=np.float32)
        wr = np.concatenate([Wr, Wr[:, e : e + 1], zcol], axis=1)
        mbrow = np.concatenate(
            [br, np.full(2, -1e30, np.float32)]
        ).astype(np.float32)
        mbrow[e] = -1e30
        mbv = np.tile(mbrow, G)
        in_maps.append(
            {
                "xg_t": xg_t,
                "w": np.ascontiguousarray(W[e]).astype(np_mm),
                "wr": np.ascontiguousarray(wr).astype(np_mm),
                "mb": np.broadcast_to(mbv[None, :], (P, G * EW)).copy(),
                "brown": np.full((P, 1), br[e], dtype=np.float32),
            }
        )
    return in_maps


def _prep(inputs):
    x = np.asarray(inputs["x"], dtype=np.float32)
    Wr = np.asarray(inputs["Wr"], dtype=np.float32)
    br = np.asarray(inputs["br"], dtype=np.float32)
    W = np.asarray(inputs["W"], dtype=np.float32)
    b = np.asarray(inputs["b"], dtype=np.float32)
    i1, i2 = _route(x, Wr, br)
    idx_per_e = [np.where((i1 == e) | (i2 == e))[0] for e in range(N_EXPERTS)]
    gp = G * P  # pad so T divides by G (whole groups only; gt=1 tails fault)
    C = max(gp, ((max(len(ix) for ix in idx_per_e) + gp - 1) // gp) * gp)
    in_maps = _make_in_maps(x, Wr, br, W, b, idx_per_e, C)
    return in_maps, idx_per_e, C, x.shape[0], b


def kernel(**inputs) -> np.ndarray:
    global LAST_RESULTS
    in_maps, idx_per_e, C, n_tokens, b = _prep(inputs)
    T = C // P
    nc = _build(C)
    res = run_bass_kernel_spmd(nc, in_maps, core_ids=list(range(N_CORES)))
    LAST_RESULTS = res

    out = np.zeros((n_tokens, D), dtype=np.float32)
    for e in range(N_CORES):
        idx = idx_per_e[e]
        n = len(idx)
        # y [P, T*D]: y[p, t*D + f] = token (t*128+p), feature f
        ye = (
            res.results[e]["y"].reshape(P, T, D).transpose(1, 0, 2).reshape(C, D)
        )
        ge = res.results[e]["gout"].T.reshape(C)
        out[idx] += ye[:n] + ge[:n, None] * b[e][None, :]
    return out
